# revision 6
# baseline (speedup 1.0000x reference)
"""EnhancedMACDCell forward on 8 Trainium2 NeuronCores.

The reference computes, per batch row b of price_series [B, 64]:
    macd[b, j]  = w_fast . price[b, e-12:e] - w_slow . price[b, e-26:e]
                  + (b_fast - b_slow),        e = 64 - 8 + j, j = 0..8
    signal[b]   = w_sig . macd[b, :] + b_sig
    hist[b]     = macd[b, 8] - signal[b]
    out[b]      = tanh(hist[b] * norm_scale + norm_bias)

Everything before the tanh is linear in price_series, so the whole model
collapses to a single 64-tap linear functional per row:
    out[b] = tanh(price[b, :] . u + c0)
with u / c0 computed on the host (float64) from the tiny weight inputs.
Only columns 30..63 of u are nonzero, so the on-device compute is a
34-wide weighted row reduction + tanh over [1M, 64] float32 - purely
memory bound (32 MiB of HBM reads per core).

Sharding: pure data parallel - 8 equal batch shards, weights replicated.
"""

import os
import sys

import numpy as np

for _p in ("/opt/trn_rl_repo", "/root/.axon_site/_ro/trn_rl_repo"):
    if os.path.isdir(_p) and _p not in sys.path:
        sys.path.insert(0, _p)

import concourse.bacc as bacc
import concourse.bass as bass
import concourse.mybir as mybir
from concourse import tile
from concourse.bass_utils import run_bass_kernel_spmd

FAST, SLOW, SIG = 12, 26, 9
S = 64
N_CORES = 8
P = 128           # SBUF partitions
R = 64            # batch rows packed per partition per tile
C_LO, C_HI = 30, 64
C = C_HI - C_LO   # 34 columns with nonzero weight


def _collapsed_weights(w_fast, b_fast, w_slow, b_slow, w_sig, b_sig,
                       norm_scale, norm_bias):
    """Fold the whole linear pipeline into (u[64], c0)."""
    wf = np.asarray(w_fast, np.float64).reshape(-1)
    ws = np.asarray(w_slow, np.float64).reshape(-1)
    wg = np.asarray(w_sig, np.float64).reshape(-1)
    A = np.zeros((SIG, S), np.float64)
    for j in range(SIG):
        e = S - (SIG - 1) + j
        A[j, e - FAST:e] += wf
        A[j, e - SLOW:e] -= ws
    coeff = -wg.copy()
    coeff[SIG - 1] += 1.0
    u = coeff @ A
    c0 = (float(np.asarray(b_fast).reshape(-1)[0])
          - float(np.asarray(b_slow).reshape(-1)[0])) * coeff.sum() \
        - float(np.asarray(b_sig).reshape(-1)[0])
    ns = float(np.asarray(norm_scale).reshape(-1)[0])
    nb = float(np.asarray(norm_bias).reshape(-1)[0])
    return (u * ns).astype(np.float32), float(c0 * ns + nb)


def _build(b_core: int, c0: float) -> bass.Bass:
    nc = bacc.Bacc()
    x = nc.declare_dram_parameter("x", [b_core, S], mybir.dt.float32,
                                  isOutput=False)
    w = nc.declare_dram_parameter("w", [P, C], mybir.dt.float32,
                                  isOutput=False)
    y = nc.declare_dram_parameter("y", [b_core], mybir.dt.float32,
                                  isOutput=True)

    rows_per_tile = P * R
    n_tiles = b_core // rows_per_tile
    assert b_core % rows_per_tile == 0

    xv = x[:].rearrange("(n p r) s -> n p r s", p=P, r=R)
    yv = y[:].rearrange("(n p r) -> n p r", p=P, r=R)

    with tile.TileContext(nc) as tc:
        with (
            tc.tile_pool(name="wp", bufs=1) as wp,
            tc.tile_pool(name="xp", bufs=3) as xp,
            tc.tile_pool(name="pp", bufs=2) as pp,
            tc.tile_pool(name="rp", bufs=2) as rp,
            tc.tile_pool(name="op", bufs=2) as op,
        ):
            wt = wp.tile([P, C], mybir.dt.float32)
            nc.sync.dma_start(wt[:], w[:])
            bt = wp.tile([P, 1], mybir.dt.float32, tag="bias")
            nc.vector.memset(bt[:], c0)
            for i in range(n_tiles):
                xt = xp.tile([P, R * S], mybir.dt.float32)
                x3 = xt[:].rearrange("p (r s) -> p r s", s=S)
                nc.sync.dma_start(x3, xv[i])
                pt = pp.tile([P, R * C], mybir.dt.float32)
                p3 = pt[:].rearrange("p (r c) -> p r c", c=C)
                wb = wt[:].unsqueeze(1).broadcast_to([P, R, C])
                nc.vector.tensor_mul(p3, x3[:, :, C_LO:C_HI], wb)
                rt = rp.tile([P, R], mybir.dt.float32)
                nc.vector.reduce_sum(rt[:], p3, axis=mybir.AxisListType.X)
                ot = op.tile([P, R], mybir.dt.float32)
                nc.scalar.activation(ot[:], rt[:],
                                     mybir.ActivationFunctionType.Tanh,
                                     bias=bt[:, 0:1], scale=1.0)
                nc.sync.dma_start(yv[i], ot[:])
    nc.compile()
    return nc


def kernel(**inputs) -> np.ndarray:
    price = np.ascontiguousarray(np.asarray(inputs["price_series"],
                                            dtype=np.float32))
    B = price.shape[0]
    assert B % N_CORES == 0
    b_core = B // N_CORES

    u, c0 = _collapsed_weights(
        inputs["w_fast"], inputs["b_fast"], inputs["w_slow"],
        inputs["b_slow"], inputs["w_sig"], inputs["b_sig"],
        inputs["norm_scale"], inputs["norm_bias"])
    w_rep = np.ascontiguousarray(
        np.broadcast_to(u[C_LO:C_HI][None, :], (P, C)))

    nc = _build(b_core, c0)
    in_maps = [
        {"x": price[i * b_core:(i + 1) * b_core], "w": w_rep}
        for i in range(N_CORES)
    ]
    res = run_bass_kernel_spmd(nc, in_maps, list(range(N_CORES)))
    out = np.concatenate([res.results[i]["y"].reshape(-1)
                          for i in range(N_CORES)])
    return out.reshape(B, 1).astype(np.float32)
